# revision 6
# baseline (speedup 1.0000x reference)
"""Trainium2 Bass kernel for nn_HRP_45664092291289.

Model: 100 independent per-zone GRU(6->64)+MLP actor units over B=2048, T=2,
followed by a "buggy stack/view" scramble p = transpose(a,(2,0,1)).reshape(B,T,N)
and a tiny critic (LocalizedModule MLP + SubCritic GRU) on zones {99,89,98}.

Sharding: the scramble is a flat reinterpretation, so contiguous chunks of the
flat index space [0, B*T*N) correspond BOTH to contiguous (unit, batch) source
blocks and contiguous output-batch blocks.  Core k gets flat chunk
[51200k, 51200(k+1)) = 25 "half-unit tasks" (unit j//2, batch-half j%2) for
j in [25k, 25k+25) = output batches [256k, 256k+256).  Everything (actor,
scramble, critic) is then core-local: pure data parallel, no collectives.

Device kernel (SPMD, one program, 8 cores):
  - tasks processed in pairs packed on the 128 SBUF partitions (A rows 0-63,
    B rows 64-127) via block-diagonal stationary weights.
  - GRU z-gate is negated host-side (z' = 1-z = sigmoid(-pre)) so
    h_new = h + z'*(n - h) everywhere.
  - All biases folded into ACT activation bias operands host-side.
  - Final actor affine 3*(sig-0.5) folded into host post-processing and the
    critic's first-layer weights (linear fold), so the device stores raw
    sigmoid values.
  - Matmuls run as float32r (full PE rate at free>=256) via bitcast.
"""

import os

import numpy as np

N_CORES = 8
B, T, N, S, H, HC, LMH = 2048, 2, 100, 6, 64, 64, 128
TASKS_PER_CORE = 25  # half-units (unit, 1024-batch half)
PAIRS = 13           # 25 tasks padded to 26 = 13 pairs
TB = 1024            # batches per task
CH = 512             # free-dim chunk
BL = 256             # critic batches per core
USE_FP32R = os.environ.get("BASS_NO_FP32R", "") == ""

_SLOTS = (6, 13, 34)  # p-feature columns inside the 35-dim critic input


# ----------------------------------------------------------------------------
# Host-side input prep
# ----------------------------------------------------------------------------

def _sig(v):
    return 1.0 / (1.0 + np.exp(-v))


def _prep_core(k, x, aw, cw):
    """Build the in_map for core k.  aw/cw: actor/critic weight dicts."""
    f32 = np.float32
    # ---- actor: 26 padded tasks -> 13 pairs ----
    # task j (global 25k+j): unit u = J//2, batch half hb = J%2
    tasks = []
    for j in range(TASKS_PER_CORE + 1):
        Jg = 25 * k + min(j, TASKS_PER_CORE - 1)  # pad task 25 = dup of 24
        tasks.append((Jg // 2, Jg % 2))

    xp = np.zeros((PAIRS, 12, T, TB), f32)
    wih = np.zeros((12, PAIRS, 3, 128), f32)
    whh = np.zeros((128, PAIRS, 3, 128), f32)
    w12 = np.zeros((128, PAIRS, 2, 128), f32)
    w3 = np.zeros((128, PAIRS, 2), f32)
    bias = np.zeros((128, PAIRS, 6), f32)
    b3 = np.zeros((2, PAIRS), f32)

    # gate order in stacked 3H tensors: r, z, n (PyTorch GRU)
    for p in range(PAIRS):
        for half in range(2):  # 0 = partitions 0-63, 1 = 64-127
            u, hb = tasks[2 * p + half]
            r0, r1 = 64 * half, 64 * half + 64
            bsl = slice(1024 * hb, 1024 * hb + 1024)
            # x: [B,T,N,S] -> [feat, t, b]
            xp[p, 6 * half:6 * half + 6] = x[bsl, :, u, :].transpose(2, 1, 0)
            Wih = aw["a_Wih"][u]  # [192, 6]
            Whh = aw["a_Whh"][u]  # [192, 64]
            bih, bhh = aw["a_bih"][u], aw["a_bhh"][u]
            for g, sgn in ((0, 1.0), (1, -1.0), (2, 1.0)):
                gs = slice(64 * g, 64 * g + 64)
                # lhsT[k, m] = W[g*64+m, k]
                wih[6 * half:6 * half + 6, p, g, r0:r1] = sgn * Wih[gs].T
                whh[r0:r1, p, g, r0:r1] = sgn * Whh[gs].T
            bias[r0:r1, p, 0] = bih[0:64] + bhh[0:64]            # r sig bias
            bias[r0:r1, p, 1] = -(bih[64:128] + bhh[64:128])     # z' sig bias
            bias[r0:r1, p, 2] = bih[128:192]                     # n tanh bias
            bias[r0:r1, p, 3] = bhh[128:192]                     # bhh_n scalar
            bias[r0:r1, p, 4] = aw["a_b1"][u]
            bias[r0:r1, p, 5] = aw["a_b2"][u]
            w12[r0:r1, p, 0, r0:r1] = aw["a_W1"][u].T
            w12[r0:r1, p, 1, r0:r1] = aw["a_W2"][u].T
            w3[r0:r1, p, half] = aw["a_W3"][u, 0]
            b3[half, p] = aw["a_b3"][u, 0]

    # ---- critic (shared weights; xi_base per core) ----
    bsl = slice(BL * k, BL * k + BL)
    xib = np.zeros((35, T, BL), f32)
    xib[0:6] = x[bsl, :, 99, :].transpose(2, 1, 0)
    xib[7:13] = x[bsl, :, 89, :].transpose(2, 1, 0)
    xib[28:34] = x[bsl, :, 98, :].transpose(2, 1, 0)

    def fold(W, b):
        # xi true p-cols hold raw sigmoid s; p = 3s - 1.5
        W = W.copy()
        adj = np.zeros(W.shape[0], W.dtype)
        for c in _SLOTS:
            adj += -1.5 * W[:, c]
            W[:, c] = 3.0 * W[:, c]
        return W, b + adj

    scWih, adj_bih = fold(cw["sc_Wih"], cw["sc_bih"])  # [192,35]
    lmW1, lm_b1 = fold(cw["lm_W1"], cw["lm_b1"])       # [128,35]

    cwih = np.zeros((35, 3, 64), f32)
    cwhh = np.zeros((64, 3, 64), f32)
    cb = np.zeros((128, 9), f32)
    bihc, bhhc = adj_bih, cw["sc_bhh"]
    for g, sgn in ((0, 1.0), (1, -1.0), (2, 1.0)):
        gs = slice(64 * g, 64 * g + 64)
        cwih[:, g, :] = sgn * scWih[gs].T
        cwhh[:, g, :] = sgn * cw["sc_Whh"][gs].T
    cb[0:64, 0] = bihc[0:64] + bhhc[0:64]
    cb[0:64, 1] = -(bihc[64:128] + bhhc[64:128])
    cb[0:64, 2] = bihc[128:192]
    cb[0:64, 3] = bhhc[128:192]
    cb[0:64, 4] = cw["sc_b1"]
    cb[0:64, 5] = cw["sc_b2"]
    cb[:, 6] = lm_b1
    cb[:, 7] = cw["lm_b2"]
    cb[0, 8] = cw["sc_b3"][0] + cw["lm_b3"][0]

    return {
        "xp": xp, "wih": wih, "whh": whh, "w12": w12, "w3": w3,
        "bias": bias, "b3": b3,
        "xib": xib, "cwih": cwih, "cwhh": cwhh, "cb": cb,
        "clm1": np.ascontiguousarray(lmW1.T).astype(f32),
        "clm2": np.ascontiguousarray(cw["lm_W2"].T).astype(f32),
        "clm3": np.ascontiguousarray(cw["lm_W3"].T).astype(f32),
        "cm1": np.ascontiguousarray(cw["sc_W1"].T).astype(f32),
        "cm2": np.ascontiguousarray(cw["sc_W2"].T).astype(f32),
        "cm3": np.ascontiguousarray(cw["sc_W3"].T).astype(f32),
    }


def _prep(inputs):
    x = np.asarray(inputs["x"], np.float32)
    aw = {n: np.asarray(inputs[n], np.float32) for n in
          ("a_Wih", "a_Whh", "a_bih", "a_bhh", "a_W1", "a_b1", "a_W2",
           "a_b2", "a_W3", "a_b3")}
    cw = {n: np.asarray(inputs[n], np.float32) for n in
          ("lm_W1", "lm_b1", "lm_W2", "lm_b2", "lm_W3", "lm_b3",
           "sc_Wih", "sc_Whh", "sc_bih", "sc_bhh", "sc_W1", "sc_b1",
           "sc_W2", "sc_b2", "sc_W3", "sc_b3")}
    return [_prep_core(k, x, aw, cw) for k in range(N_CORES)]


def _post(results):
    a_full = np.concatenate([r["a_out"].reshape(-1) for r in results])
    p_flat = 3.0 * a_full - 1.5
    out1 = p_flat.reshape(B, T, N)[:, 1, :].copy()
    out2 = np.concatenate([r["q_out"].reshape(-1) for r in results])
    return out1.astype(np.float32), out2.reshape(-1, 1).astype(np.float32)


# ----------------------------------------------------------------------------
# Numpy emulation of the exact device program (for logic validation)
# ----------------------------------------------------------------------------

def _emulate_core(m):
    f32 = np.float32
    a_out = np.zeros(TASKS_PER_CORE * TB * T, f32)
    av2 = a_out.reshape(25, TB, 2)
    for p in range(PAIRS):
        for c in range(2):
            xt = m["xp"][p, :, :, CH * c:CH * c + CH]  # [12, 2, 512]
            gx = [m["wih"][:, p, g, :].T @ xt.reshape(12, -1)
                  for g in range(3)]  # each [128, 1024] (t0|t1)
            gx = [g.reshape(128, 2, CH) for g in gx]
            r0 = _sig(gx[0][:, 0] + m["bias"][:, p, 0:1])
            z0 = _sig(gx[1][:, 0] + m["bias"][:, p, 1:2])
            n0 = np.tanh(gx[2][:, 0] + r0 * m["bias"][:, p, 3:4]
                         + m["bias"][:, p, 2:3])
            h1 = z0 * n0
            a0 = np.maximum(h1, 0)
            gh = [m["whh"][:, p, g, :].T @ h1 for g in range(3)]
            r1 = _sig(gx[0][:, 1] + gh[0] + m["bias"][:, p, 0:1])
            z1 = _sig(gx[1][:, 1] + gh[1] + m["bias"][:, p, 1:2])
            hn = gh[2] + m["bias"][:, p, 3:4]
            n1 = np.tanh(gx[2][:, 1] + r1 * hn + m["bias"][:, p, 2:3])
            h2 = h1 + z1 * (n1 - h1)
            a1p = np.maximum(h2, 0)
            for t, a_in in ((0, a0), (1, a1p)):
                a1 = np.maximum(m["w12"][:, p, 0, :].T @ a_in
                                + m["bias"][:, p, 4:5], 0)
                a2 = np.maximum(m["w12"][:, p, 1, :].T @ a1
                                + m["bias"][:, p, 5:6], 0)
                sg = _sig(m["w3"][:, p, :].T @ a2 + m["b3"][:, p:p + 1])
                for half in range(2):
                    jl = 2 * p + half
                    if jl >= TASKS_PER_CORE:
                        continue
                    av2[jl, CH * c:CH * c + CH, t] = sg[half]
    # ---- critic ----
    av200 = a_out.reshape(BL, 200)
    xi = m["xib"].copy()
    for row, n in ((6, 99), (13, 89), (34, 98)):
        for t in range(2):
            xi[row, t] = av200[:, 100 * t + n]
    cb = m["cb"]
    gx = [m["cwih"][:, g, :].T @ xi.reshape(35, -1) for g in range(3)]
    gx = [g.reshape(64, 2, BL) for g in gx]
    r0 = _sig(gx[0][:, 0] + cb[0:64, 0:1])
    z0 = _sig(gx[1][:, 0] + cb[0:64, 1:2])
    n0 = np.tanh(gx[2][:, 0] + r0 * cb[0:64, 3:4] + cb[0:64, 2:3])
    h1 = z0 * n0
    gh = [m["cwhh"][:, g, :].T @ h1 for g in range(3)]
    r1 = _sig(gx[0][:, 1] + gh[0] + cb[0:64, 0:1])
    z1 = _sig(gx[1][:, 1] + gh[1] + cb[0:64, 1:2])
    n1 = np.tanh(gx[2][:, 1] + r1 * (gh[2] + cb[0:64, 3:4]) + cb[0:64, 2:3])
    h2 = h1 + z1 * (n1 - h1)
    ca = np.maximum(h2, 0)
    ca1 = np.maximum(m["cm1"].T @ ca + cb[0:64, 4:5], 0)
    ca2 = np.maximum(m["cm2"].T @ ca1 + cb[0:64, 5:6], 0)
    q3 = m["cm3"].T @ ca2  # [1, 256]
    f1 = np.maximum(m["clm1"].T @ xi[:, 1] + cb[:, 6:7], 0)
    f2 = np.maximum(m["clm2"].T @ f1 + cb[:, 7:8], 0)
    f3 = m["clm3"].T @ f2
    q_out = (q3 + f3 + cb[0, 8])[0]
    return {"a_out": a_out, "q_out": q_out.astype(f32)}


# ----------------------------------------------------------------------------
# Bass device program
# ----------------------------------------------------------------------------

_CACHE = {}


def _build_program():
    import concourse.bass as bass
    import concourse.mybir as mybir
    import concourse.tile as tile
    from concourse import bacc
    from contextlib import ExitStack

    f32 = mybir.dt.float32
    f32r = mybir.dt.float32r

    nc = bacc.Bacc("TRN2", target_bir_lowering=False, debug=False,
                   num_devices=N_CORES)
    mdt = f32r if USE_FP32R else f32

    d_xp = nc.dram_tensor("xp", (PAIRS, 12, T, TB), mdt, kind="ExternalInput")
    d_wih = nc.dram_tensor("wih", (12, PAIRS, 3, 128), mdt, kind="ExternalInput")
    d_whh = nc.dram_tensor("whh", (128, PAIRS, 3, 128), mdt, kind="ExternalInput")
    d_w12 = nc.dram_tensor("w12", (128, PAIRS, 2, 128), mdt, kind="ExternalInput")
    d_w3 = nc.dram_tensor("w3", (128, PAIRS, 2), mdt, kind="ExternalInput")
    d_bias = nc.dram_tensor("bias", (128, PAIRS, 6), f32, kind="ExternalInput")
    d_b3 = nc.dram_tensor("b3", (2, PAIRS), f32, kind="ExternalInput")
    d_xib = nc.dram_tensor("xib", (35, T, BL), mdt, kind="ExternalInput")
    d_cwih = nc.dram_tensor("cwih", (35, 3, 64), mdt, kind="ExternalInput")
    d_cwhh = nc.dram_tensor("cwhh", (64, 3, 64), mdt, kind="ExternalInput")
    d_cb = nc.dram_tensor("cb", (128, 9), f32, kind="ExternalInput")
    d_clm1 = nc.dram_tensor("clm1", (35, 128), mdt, kind="ExternalInput")
    d_clm2 = nc.dram_tensor("clm2", (128, 128), mdt, kind="ExternalInput")
    d_clm3 = nc.dram_tensor("clm3", (128, 1), mdt, kind="ExternalInput")
    d_cm1 = nc.dram_tensor("cm1", (64, 64), mdt, kind="ExternalInput")
    d_cm2 = nc.dram_tensor("cm2", (64, 64), mdt, kind="ExternalInput")
    d_cm3 = nc.dram_tensor("cm3", (64, 1), mdt, kind="ExternalInput")

    d_aout = nc.dram_tensor("a_out", (TASKS_PER_CORE * TB * T,), f32,
                            kind="ExternalOutput")
    d_qout = nc.dram_tensor("q_out", (BL,), f32, kind="ExternalOutput")
    d_dummy = nc.dram_tensor("pad_sink", (TB, 2), f32, kind="Internal")

    Sig = mybir.ActivationFunctionType.Sigmoid
    Tanh = mybir.ActivationFunctionType.Tanh
    Relu = mybir.ActivationFunctionType.Relu
    ADD = mybir.AluOpType.add
    MAX = mybir.AluOpType.max

    mdt = f32r if USE_FP32R else f32

    def mmc(ap):
        return ap

    with tile.TileContext(nc) as tc, ExitStack() as ctx:
        res = ctx.enter_context(tc.tile_pool(name="res", bufs=1))
        s_wih = res.tile([12, PAIRS, 3, 128], mdt)
        s_whh = res.tile([128, PAIRS, 3, 128], mdt)
        s_w12 = res.tile([128, PAIRS, 2, 128], mdt)
        s_w3 = res.tile([128, PAIRS, 2], mdt)
        s_bias = res.tile([128, PAIRS, 6], f32)
        s_b3 = res.tile([2, PAIRS], f32)
        for s, d in ((s_wih, d_wih), (s_whh, d_whh), (s_w12, d_w12),
                     (s_w3, d_w3), (s_bias, d_bias), (s_b3, d_b3)):
            nc.sync.dma_start(out=s, in_=d.ap())

        av3 = d_aout.ap().rearrange("(j b two) -> j b two", b=TB, two=2)
        av200 = d_aout.ap().rearrange("(r c) -> r c", c=200)

        with tc.tile_pool(name="wk", bufs=2) as wk, \
             tc.tile_pool(name="pr", bufs=1, space="PSUM") as pr, \
             tc.tile_pool(name="pz", bufs=1, space="PSUM") as pz, \
             tc.tile_pool(name="pn", bufs=2, space="PSUM") as pn, \
             tc.tile_pool(name="pg", bufs=1, space="PSUM") as pg, \
             tc.tile_pool(name="pm", bufs=2, space="PSUM") as pm, \
             tc.tile_pool(name="p3", bufs=1, space="PSUM") as p3:
            for p in range(PAIRS):
                w_r = s_wih[:, p, 0, :]
                w_z = s_wih[:, p, 1, :]
                w_n = s_wih[:, p, 2, :]
                b_r = s_bias[:, p, 0:1]
                b_z = s_bias[:, p, 1:2]
                b_tn = s_bias[:, p, 2:3]
                b_hn = s_bias[:, p, 3:4]
                b_1 = s_bias[:, p, 4:5]
                b_2 = s_bias[:, p, 5:6]
                for c in range(2):
                    x_t = wk.tile([12, T, CH], mdt, tag="x")
                    nc.sync.dma_start(
                        out=x_t, in_=d_xp.ap()[p, :, :, CH * c:CH * c + CH])
                    ps_r0 = pr.tile([128, CH], f32, tag="pr")
                    ps_z0 = pz.tile([128, CH], f32, tag="pz")
                    ps_n0 = pn.tile([128, CH], f32, tag="pn")
                    nc.tensor.matmul(ps_r0, mmc(w_r), mmc(x_t[:, 0, :]),
                                     start=True, stop=True)
                    nc.tensor.matmul(ps_z0, mmc(w_z), mmc(x_t[:, 0, :]),
                                     start=True, stop=True)
                    nc.tensor.matmul(ps_n0, mmc(w_n), mmc(x_t[:, 0, :]),
                                     start=True, stop=True)
                    r0 = wk.tile([128, CH], f32, tag="r0")
                    z0 = wk.tile([128, CH], f32, tag="z0")
                    nc.scalar.activation(r0, ps_r0, Sig, bias=b_r)
                    nc.scalar.activation(z0, ps_z0, Sig, bias=b_z)
                    rbn = wk.tile([128, CH], f32, tag="rbn")
                    nc.vector.tensor_scalar_mul(rbn, r0, b_hn)
                    pre0 = wk.tile([128, CH], f32, tag="pre0")
                    nc.vector.tensor_add(pre0, ps_n0, rbn)
                    n0 = wk.tile([128, CH], f32, tag="n0")
                    nc.scalar.activation(n0, pre0, Tanh, bias=b_tn)
                    h1 = wk.tile([128, CH], mdt, tag="h1")
                    nc.vector.tensor_mul(h1, z0, n0)
                    a0 = wk.tile([128, CH], mdt, tag="a0")
                    nc.vector.tensor_scalar_max(a0, h1, 0.0)

                    ps_r1 = pr.tile([128, CH], f32, tag="pr")
                    ps_z1 = pz.tile([128, CH], f32, tag="pz")
                    ps_n1 = pn.tile([128, CH], f32, tag="pn")
                    ps_gn = pg.tile([128, CH], f32, tag="pg")
                    nc.tensor.matmul(ps_r1, mmc(w_r), mmc(x_t[:, 1, :]),
                                     start=True, stop=False)
                    nc.tensor.matmul(ps_z1, mmc(w_z), mmc(x_t[:, 1, :]),
                                     start=True, stop=False)
                    nc.tensor.matmul(ps_n1, mmc(w_n), mmc(x_t[:, 1, :]),
                                     start=True, stop=True)
                    nc.tensor.matmul(ps_r1, mmc(s_whh[:, p, 0, :]), mmc(h1),
                                     start=False, stop=True)
                    nc.tensor.matmul(ps_z1, mmc(s_whh[:, p, 1, :]), mmc(h1),
                                     start=False, stop=True)
                    nc.tensor.matmul(ps_gn, mmc(s_whh[:, p, 2, :]), mmc(h1),
                                     start=True, stop=True)
                    r1 = wk.tile([128, CH], f32, tag="r0")
                    z1 = wk.tile([128, CH], f32, tag="z0")
                    nc.scalar.activation(r1, ps_r1, Sig, bias=b_r)
                    nc.scalar.activation(z1, ps_z1, Sig, bias=b_z)
                    hn = wk.tile([128, CH], f32, tag="rbn")
                    nc.vector.tensor_scalar_add(hn, ps_gn, b_hn)
                    rhn = wk.tile([128, CH], f32, tag="rhn")
                    nc.vector.tensor_mul(rhn, r1, hn)
                    pre1 = wk.tile([128, CH], f32, tag="pre0")
                    nc.vector.tensor_add(pre1, ps_n1, rhn)
                    n1 = wk.tile([128, CH], f32, tag="n0")
                    nc.scalar.activation(n1, pre1, Tanh, bias=b_tn)
                    dd = wk.tile([128, CH], f32, tag="dd")
                    nc.vector.tensor_sub(dd, n1, h1)
                    zd = wk.tile([128, CH], f32, tag="zd")
                    nc.vector.tensor_mul(zd, z1, dd)
                    h2 = wk.tile([128, CH], f32, tag="h1b")
                    nc.vector.tensor_add(h2, h1, zd)
                    a1p = wk.tile([128, CH], mdt, tag="a1p")
                    nc.vector.tensor_scalar_max(a1p, h2, 0.0)

                    for t, a_in in ((0, a0), (1, a1p)):
                        ps_m1 = pm.tile([128, CH], f32, tag="pm")
                        nc.tensor.matmul(ps_m1, mmc(s_w12[:, p, 0, :]),
                                         mmc(a_in), start=True, stop=True)
                        a1 = wk.tile([128, CH], mdt, tag="a1")
                        nc.scalar.activation(a1, ps_m1, Relu, bias=b_1)
                        ps_m2 = pm.tile([128, CH], f32, tag="pm")
                        nc.tensor.matmul(ps_m2, mmc(s_w12[:, p, 1, :]),
                                         mmc(a1), start=True, stop=True)
                        a2 = wk.tile([128, CH], mdt, tag="a2")
                        nc.vector.tensor_scalar(a2, ps_m2, b_2, 0.0, ADD, MAX)
                        ps_m3 = p3.tile([2, CH], f32, tag="p3")
                        nc.tensor.matmul(ps_m3, mmc(s_w3[:, p, :]), mmc(a2),
                                         start=True, stop=True)
                        sg = wk.tile([2, CH], f32, tag="sg")
                        nc.scalar.activation(sg, ps_m3, Sig,
                                             bias=s_b3[:, p:p + 1])
                        if p < PAIRS - 1:
                            nc.sync.dma_start(
                                out=av3[2 * p:2 * p + 2,
                                        CH * c:CH * c + CH, t],
                                in_=sg)
                        else:
                            nc.sync.dma_start(
                                out=av3[2 * p:2 * p + 1,
                                        CH * c:CH * c + CH, t],
                                in_=sg[0:1, :])
                            nc.sync.dma_start(
                                out=d_dummy.ap()[CH * c:CH * c + CH, t],
                                in_=sg[1:2, :])

        # ------------------ critic ------------------
        with tc.tile_pool(name="cw", bufs=1) as cwp, \
             tc.tile_pool(name="ck", bufs=2) as ck, \
             tc.tile_pool(name="cp", bufs=1, space="PSUM") as cp, \
             tc.tile_pool(name="cpf", bufs=1, space="PSUM") as cpf:
            s_cwih = cwp.tile([35, 3, 64], mdt)
            s_cwhh = cwp.tile([64, 3, 64], mdt)
            s_cb = cwp.tile([128, 9], f32)
            s_clm1 = cwp.tile([35, 128], mdt)
            s_clm2 = cwp.tile([128, 128], mdt)
            s_clm3 = cwp.tile([128, 1], mdt)
            s_cm1 = cwp.tile([64, 64], mdt)
            s_cm2 = cwp.tile([64, 64], mdt)
            s_cm3 = cwp.tile([64, 1], mdt)
            for s, d in ((s_cwih, d_cwih), (s_cwhh, d_cwhh), (s_cb, d_cb),
                         (s_clm1, d_clm1), (s_clm2, d_clm2), (s_clm3, d_clm3),
                         (s_cm1, d_cm1), (s_cm2, d_cm2), (s_cm3, d_cm3)):
                nc.sync.dma_start(out=s, in_=d.ap())
            xi = cwp.tile([35, T, BL], mdt)
            nc.sync.dma_start(out=xi, in_=d_xib.ap())
            for row, n in ((6, 99), (13, 89), (34, 98)):
                for t in range(2):
                    nc.gpsimd.dma_start(out=xi[row:row + 1, t, :],
                                        in_=av200[:, 100 * t + n:100 * t + n + 1])

            cb_r = s_cb[0:64, 0:1]
            cb_z = s_cb[0:64, 1:2]
            cb_tn = s_cb[0:64, 2:3]
            cb_hn = s_cb[0:64, 3:4]

            ps_cr0 = cp.tile([64, BL], f32, tag="cr")
            ps_cz0 = cp.tile([64, BL], f32, tag="cz")
            ps_cn0 = cp.tile([64, BL], f32, tag="cn")
            nc.tensor.matmul(ps_cr0, mmc(s_cwih[:, 0, :]), mmc(xi[:, 0, :]),
                             start=True, stop=True)
            nc.tensor.matmul(ps_cz0, mmc(s_cwih[:, 1, :]), mmc(xi[:, 0, :]),
                             start=True, stop=True)
            nc.tensor.matmul(ps_cn0, mmc(s_cwih[:, 2, :]), mmc(xi[:, 0, :]),
                             start=True, stop=True)
            cr0 = ck.tile([64, BL], f32, tag="cr0")
            cz0 = ck.tile([64, BL], f32, tag="cz0")
            nc.scalar.activation(cr0, ps_cr0, Sig, bias=cb_r)
            nc.scalar.activation(cz0, ps_cz0, Sig, bias=cb_z)
            crbn = ck.tile([64, BL], f32, tag="crbn")
            nc.vector.tensor_scalar_mul(crbn, cr0, cb_hn)
            cpre0 = ck.tile([64, BL], f32, tag="cpre0")
            nc.vector.tensor_add(cpre0, ps_cn0, crbn)
            cn0 = ck.tile([64, BL], f32, tag="cn0")
            nc.scalar.activation(cn0, cpre0, Tanh, bias=cb_tn)
            ch1 = ck.tile([64, BL], mdt, tag="ch1")
            nc.vector.tensor_mul(ch1, cz0, cn0)

            ps_cr1 = cp.tile([64, BL], f32, tag="cr")
            ps_cz1 = cp.tile([64, BL], f32, tag="cz")
            ps_cn1 = cp.tile([64, BL], f32, tag="cn")
            ps_cg = cp.tile([64, BL], f32, tag="cg")
            nc.tensor.matmul(ps_cr1, mmc(s_cwih[:, 0, :]), mmc(xi[:, 1, :]),
                             start=True, stop=False)
            nc.tensor.matmul(ps_cz1, mmc(s_cwih[:, 1, :]), mmc(xi[:, 1, :]),
                             start=True, stop=False)
            nc.tensor.matmul(ps_cn1, mmc(s_cwih[:, 2, :]), mmc(xi[:, 1, :]),
                             start=True, stop=True)
            nc.tensor.matmul(ps_cr1, mmc(s_cwhh[:, 0, :]), mmc(ch1),
                             start=False, stop=True)
            nc.tensor.matmul(ps_cz1, mmc(s_cwhh[:, 1, :]), mmc(ch1),
                             start=False, stop=True)
            nc.tensor.matmul(ps_cg, mmc(s_cwhh[:, 2, :]), mmc(ch1),
                             start=True, stop=True)
            cr1 = ck.tile([64, BL], f32, tag="cr0")
            cz1 = ck.tile([64, BL], f32, tag="cz0")
            nc.scalar.activation(cr1, ps_cr1, Sig, bias=cb_r)
            nc.scalar.activation(cz1, ps_cz1, Sig, bias=cb_z)
            chn = ck.tile([64, BL], f32, tag="crbn")
            nc.vector.tensor_scalar_add(chn, ps_cg, cb_hn)
            crhn = ck.tile([64, BL], f32, tag="crhn")
            nc.vector.tensor_mul(crhn, cr1, chn)
            cpre1 = ck.tile([64, BL], f32, tag="cpre0")
            nc.vector.tensor_add(cpre1, ps_cn1, crhn)
            cn1 = ck.tile([64, BL], f32, tag="cn0")
            nc.scalar.activation(cn1, cpre1, Tanh, bias=cb_tn)
            cdd = ck.tile([64, BL], f32, tag="cdd")
            nc.vector.tensor_sub(cdd, cn1, ch1)
            czd = ck.tile([64, BL], f32, tag="czd")
            nc.vector.tensor_mul(czd, cz1, cdd)
            ch2 = ck.tile([64, BL], f32, tag="ch2")
            nc.vector.tensor_add(ch2, ch1, czd)
            ca = ck.tile([64, BL], mdt, tag="ca")
            nc.vector.tensor_scalar_max(ca, ch2, 0.0)

            ps_cm1 = cp.tile([64, BL], f32, tag="cm")
            nc.tensor.matmul(ps_cm1, mmc(s_cm1), mmc(ca),
                             start=True, stop=True)
            ca1 = ck.tile([64, BL], mdt, tag="ca1")
            nc.scalar.activation(ca1, ps_cm1, Relu, bias=s_cb[0:64, 4:5])
            ps_cm2 = cp.tile([64, BL], f32, tag="cm")
            nc.tensor.matmul(ps_cm2, mmc(s_cm2), mmc(ca1),
                             start=True, stop=True)
            ca2 = ck.tile([64, BL], mdt, tag="ca2")
            nc.scalar.activation(ca2, ps_cm2, Relu, bias=s_cb[0:64, 5:6])
            ps_cq = cpf.tile([1, BL], f32, tag="cq")
            nc.tensor.matmul(ps_cq, mmc(s_cm3), mmc(ca2),
                             start=True, stop=False)

            ps_cf1 = cp.tile([128, BL], f32, tag="cf")
            nc.tensor.matmul(ps_cf1, mmc(s_clm1), mmc(xi[:, 1, :]),
                             start=True, stop=True)
            cf1 = ck.tile([128, BL], mdt, tag="cf1")
            nc.scalar.activation(cf1, ps_cf1, Relu, bias=s_cb[:, 6:7])
            ps_cf2 = cp.tile([128, BL], f32, tag="cf")
            nc.tensor.matmul(ps_cf2, mmc(s_clm2), mmc(cf1),
                             start=True, stop=True)
            cf2 = ck.tile([128, BL], mdt, tag="cf2")
            nc.scalar.activation(cf2, ps_cf2, Relu, bias=s_cb[:, 7:8])
            nc.tensor.matmul(ps_cq, mmc(s_clm3), mmc(cf2),
                             start=False, stop=True)

            qout = ck.tile([1, BL], f32, tag="qout")
            nc.vector.tensor_scalar_add(qout, ps_cq, s_cb[0:1, 8:9])
            nc.sync.dma_start(out=d_qout.ap(), in_=qout)

    nc.compile()
    return nc


def kernel(**inputs):
    in_maps = _prep(inputs)
    if os.environ.get("BASS_KERNEL_EMULATE", ""):
        results = [_emulate_core(m) for m in in_maps]
        return _post(results)

    if "nc" not in _CACHE:
        _CACHE["nc"] = _build_program()
    nc = _CACHE["nc"]

    from concourse import bass_utils
    trace = bool(os.environ.get("BASS_KERNEL_TRACE", ""))
    tmpdir = os.environ.get("BASS_KERNEL_TMPDIR") or None
    r = bass_utils.run_bass_kernel_spmd(
        nc, in_maps, core_ids=list(range(N_CORES)), trace=trace,
        tmpdir=tmpdir)
    _CACHE["last_exec_time_ns"] = r.exec_time_ns
    return _post(r.results)


# revision 11
# speedup vs baseline: 440.7597x; 440.7597x over previous
"""Trainium2 Bass kernel for nn_HRP_45664092291289.

Model: 100 independent per-zone GRU(6->64)+MLP actor units over B=2048, T=2,
followed by a "buggy stack/view" scramble p = transpose(a,(2,0,1)).reshape(B,T,N)
and a tiny critic (LocalizedModule MLP + SubCritic GRU) on zones {99,89,98}.

Sharding: the scramble is a flat reinterpretation, so contiguous chunks of the
flat index space [0, B*T*N) correspond BOTH to contiguous (unit, batch) source
blocks and contiguous output-batch blocks.  Core k gets flat chunk
[51200k, 51200(k+1)) = 25 "half-unit tasks" (unit j//2, batch-half j%2) for
j in [25k, 25k+25) = output batches [256k, 256k+256).  Everything (actor,
scramble, critic) is then core-local: pure data parallel, no collectives.

Device kernel (SPMD, one program, 8 cores):
  - tasks processed in pairs packed on the 128 SBUF partitions (A rows 0-63,
    B rows 64-127) via block-diagonal stationary weights.
  - GRU z-gate is negated host-side (z' = 1-z = sigmoid(-pre)) so
    h_new = h + z'*(n - h) everywhere.
  - All biases folded into ACT activation bias operands host-side.
  - Device stores RAW final-layer pre-activations; host applies
    sigmoid/affine (p = 3*sig(v + b3) - 1.5); the critic applies
    sigmoid on-device only to the 6 gathered p-rows, with the affine
    folded into the critic first-layer weights.
  - gx matmuls fp32r; recurrent/MLP matmuls and elementwise chain bf16.
"""

import os

import numpy as np

N_CORES = 8
B, T, N, S, H, HC, LMH = 2048, 2, 100, 6, 64, 64, 128
TASKS_PER_CORE = 25  # half-units (unit, 1024-batch half)
PAIRS = 13           # 25 tasks padded to 26 = 13 pairs
TB = 1024            # batches per task
CH = 512             # free-dim chunk
BL = 256             # critic batches per core
USE_FP32R = os.environ.get("BASS_NO_FP32R", "") == ""
ELT_BF16 = os.environ.get("BASS_NO_BF16", "") == ""

# critic feature reorder: x-features first, the 3 p-slots contiguous at 32-34
# orig layout: [x99(0-5), p99(6), x89(7-12), p89(13), zeros(14-27),
#               x98(28-33), p98(34)]
_PERM = (list(range(0, 6)) + list(range(7, 13)) + list(range(28, 34))
         + list(range(14, 28)) + [6, 13, 34])
_PSLOT_ROWS = (32, 33, 34)
_PNS = (99, 89, 98)


def _sig(v):
    return 1.0 / (1.0 + np.exp(-v))


# ----------------------------------------------------------------------------
# Host-side input prep
# ----------------------------------------------------------------------------

def _prep_core(k, x, aw, cw):
    import ml_dtypes
    f32 = np.float32
    bf16 = ml_dtypes.bfloat16
    wdt = bf16 if ELT_BF16 else f32

    tasks = []
    for j in range(TASKS_PER_CORE + 1):
        Jg = 25 * k + min(j, TASKS_PER_CORE - 1)  # pad task 25 = dup of 24
        tasks.append((Jg // 2, Jg % 2))

    xp = np.zeros((PAIRS, 12, T, TB), f32)
    wih = np.zeros((12, PAIRS, 3, 128), f32)
    whh = np.zeros((128, PAIRS, 3, 128), f32)
    w12 = np.zeros((128, PAIRS, 2, 128), f32)
    w3 = np.zeros((128, PAIRS, 2), f32)
    bias = np.zeros((128, PAIRS, 6), f32)

    for p in range(PAIRS):
        for half in range(2):  # 0 = partitions 0-63, 1 = 64-127
            u, hb = tasks[2 * p + half]
            r0, r1 = 64 * half, 64 * half + 64
            bsl = slice(1024 * hb, 1024 * hb + 1024)
            xp[p, 6 * half:6 * half + 6] = x[bsl, :, u, :].transpose(2, 1, 0)
            Wih = aw["a_Wih"][u]  # [192, 6]
            Whh = aw["a_Whh"][u]  # [192, 64]
            bih, bhh = aw["a_bih"][u], aw["a_bhh"][u]
            for g, sgn in ((0, 1.0), (1, -1.0), (2, 1.0)):
                gs = slice(64 * g, 64 * g + 64)
                wih[6 * half:6 * half + 6, p, g, r0:r1] = sgn * Wih[gs].T
                whh[r0:r1, p, g, r0:r1] = sgn * Whh[gs].T
            bias[r0:r1, p, 0] = bih[0:64] + bhh[0:64]            # r sig bias
            bias[r0:r1, p, 1] = -(bih[64:128] + bhh[64:128])     # z' sig bias
            bias[r0:r1, p, 2] = bih[128:192]                     # n tanh bias
            bias[r0:r1, p, 3] = bhh[128:192]                     # bhh_n scalar
            bias[r0:r1, p, 4] = aw["a_b1"][u]
            bias[r0:r1, p, 5] = aw["a_b2"][u]
            w12[r0:r1, p, 0, r0:r1] = aw["a_W1"][u].T
            w12[r0:r1, p, 1, r0:r1] = aw["a_W2"][u].T
            w3[r0:r1, p, half] = aw["a_W3"][u, 0]

    # ---- critic ----
    bsl = slice(BL * k, BL * k + BL)
    xib = np.zeros((35, T, BL), f32)
    xib[0:6] = x[bsl, :, 99, :].transpose(2, 1, 0)
    xib[6:12] = x[bsl, :, 89, :].transpose(2, 1, 0)
    xib[12:18] = x[bsl, :, 98, :].transpose(2, 1, 0)

    # per-element b3 for the gathered p-rows: unit = global_flat // 4096
    b3g = np.zeros((35, T, BL), f32)
    for s, n in enumerate(_PNS):
        for t in range(T):
            gflat = 51200 * k + 200 * np.arange(BL) + 100 * t + n
            b3g[_PSLOT_ROWS[s], t] = aw["a_b3"][gflat // 4096, 0]

    def fold(W, b):
        W = W.copy()
        adj = np.zeros(W.shape[0], W.dtype)
        for c in (6, 13, 34):
            adj += -1.5 * W[:, c]
            W[:, c] = 3.0 * W[:, c]
        return W[:, _PERM], b + adj

    scWih, adj_bih = fold(cw["sc_Wih"], cw["sc_bih"])  # [192,35] permuted
    lmW1, lm_b1 = fold(cw["lm_W1"], cw["lm_b1"])       # [128,35] permuted

    cwih = np.zeros((35, 3, 64), f32)
    cwhh = np.zeros((64, 3, 64), f32)
    cb = np.zeros((128, 9), f32)
    bihc, bhhc = adj_bih, cw["sc_bhh"]
    for g, sgn in ((0, 1.0), (1, -1.0), (2, 1.0)):
        gs = slice(64 * g, 64 * g + 64)
        cwih[:, g, :] = sgn * scWih[gs].T
        cwhh[:, g, :] = sgn * cw["sc_Whh"][gs].T
    cb[0:64, 0] = bihc[0:64] + bhhc[0:64]
    cb[0:64, 1] = -(bihc[64:128] + bhhc[64:128])
    cb[0:64, 2] = bihc[128:192]
    cb[0:64, 3] = bhhc[128:192]
    cb[0:64, 4] = cw["sc_b1"]
    cb[0:64, 5] = cw["sc_b2"]
    cb[:, 6] = lm_b1
    cb[:, 7] = cw["lm_b2"]
    cb[0, 8] = cw["sc_b3"][0] + cw["lm_b3"][0]

    def c(a):
        return np.ascontiguousarray(a).astype(f32)

    return {
        "xp": xp, "wih": wih,
        "whh": whh.astype(wdt), "w12": w12.astype(wdt), "w3": w3.astype(wdt),
        "bias": bias,
        "xib": xib, "b3g": b3g, "cwih": cwih, "cwhh": cwhh, "cb": cb,
        "clm1": c(lmW1.T), "clm2": c(cw["lm_W2"].T), "clm3": c(cw["lm_W3"].T),
        "cm1": c(cw["sc_W1"].T), "cm2": c(cw["sc_W2"].T),
        "cm3": c(cw["sc_W3"].T),
    }


def _prep(inputs):
    x = np.asarray(inputs["x"], np.float32)
    aw = {n: np.asarray(inputs[n], np.float32) for n in
          ("a_Wih", "a_Whh", "a_bih", "a_bhh", "a_W1", "a_b1", "a_W2",
           "a_b2", "a_W3", "a_b3")}
    cw = {n: np.asarray(inputs[n], np.float32) for n in
          ("lm_W1", "lm_b1", "lm_W2", "lm_b2", "lm_W3", "lm_b3",
           "sc_Wih", "sc_Whh", "sc_bih", "sc_bhh", "sc_W1", "sc_b1",
           "sc_W2", "sc_b2", "sc_W3", "sc_b3")}
    return [_prep_core(k, x, aw, cw) for k in range(N_CORES)], aw


def _post(results, aw):
    a_full = np.concatenate(
        [np.asarray(r["a_out"], np.float32).reshape(-1) for r in results])
    b3_flat = np.repeat(aw["a_b3"][:, 0], B * T)  # unit = flat // 4096
    p_flat = 3.0 * _sig(a_full + b3_flat) - 1.5
    out1 = p_flat.reshape(B, T, N)[:, 1, :].copy()
    out2 = np.concatenate(
        [np.asarray(r["q_out"], np.float32).reshape(-1) for r in results])
    return out1.astype(np.float32), out2.reshape(-1, 1).astype(np.float32)


# ----------------------------------------------------------------------------
# Numpy emulation of the exact device program (for logic validation)
# ----------------------------------------------------------------------------

def _emulate_core(m):
    f32 = np.float32
    a_out = np.zeros(TASKS_PER_CORE * TB * T, f32)
    av2 = a_out.reshape(25, TB, 2)
    whh = np.asarray(m["whh"], f32)
    w12 = np.asarray(m["w12"], f32)
    w3 = np.asarray(m["w3"], f32)
    for p in range(PAIRS):
        for c in range(2):
            xt = m["xp"][p, :, :, CH * c:CH * c + CH]  # [12, 2, 512]
            gx = [m["wih"][:, p, g, :].T @ xt.reshape(12, -1)
                  for g in range(3)]
            gx = [g.reshape(128, 2, CH) for g in gx]
            r0 = _sig(gx[0][:, 0] + m["bias"][:, p, 0:1])
            z0 = _sig(gx[1][:, 0] + m["bias"][:, p, 1:2])
            n0 = np.tanh(gx[2][:, 0] + r0 * m["bias"][:, p, 3:4]
                         + m["bias"][:, p, 2:3])
            h1 = z0 * n0
            a0 = np.maximum(h1, 0)
            gh = [whh[:, p, g, :].T @ h1 for g in range(3)]
            r1 = _sig(gx[0][:, 1] + gh[0] + m["bias"][:, p, 0:1])
            z1 = _sig(gx[1][:, 1] + gh[1] + m["bias"][:, p, 1:2])
            hn = gh[2] + m["bias"][:, p, 3:4]
            n1 = np.tanh(gx[2][:, 1] + r1 * hn + m["bias"][:, p, 2:3])
            h2 = h1 + z1 * (n1 - h1)
            a1p = np.maximum(h2, 0)
            for t, a_in in ((0, a0), (1, a1p)):
                a1 = np.maximum(w12[:, p, 0, :].T @ a_in
                                + m["bias"][:, p, 4:5], 0)
                a2 = np.maximum(w12[:, p, 1, :].T @ a1
                                + m["bias"][:, p, 5:6], 0)
                raw = w3[:, p, :].T @ a2  # [2, 512] pre-bias pre-sigmoid
                for half in range(2):
                    jl = 2 * p + half
                    if jl >= TASKS_PER_CORE:
                        continue
                    av2[jl, CH * c:CH * c + CH, t] = raw[half]
    # ---- critic ----
    av200 = a_out.reshape(BL, 200)
    xi = m["xib"].copy()
    for s, n in enumerate(_PNS):
        for t in range(2):
            xi[_PSLOT_ROWS[s], t] = _sig(av200[:, 100 * t + n]
                                         + m["b3g"][_PSLOT_ROWS[s], t])
    cb = m["cb"]
    gx = [m["cwih"][:, g, :].T @ xi.reshape(35, -1) for g in range(3)]
    gx = [g.reshape(64, 2, BL) for g in gx]
    r0 = _sig(gx[0][:, 0] + cb[0:64, 0:1])
    z0 = _sig(gx[1][:, 0] + cb[0:64, 1:2])
    n0 = np.tanh(gx[2][:, 0] + r0 * cb[0:64, 3:4] + cb[0:64, 2:3])
    h1 = z0 * n0
    gh = [m["cwhh"][:, g, :].T @ h1 for g in range(3)]
    r1 = _sig(gx[0][:, 1] + gh[0] + cb[0:64, 0:1])
    z1 = _sig(gx[1][:, 1] + gh[1] + cb[0:64, 1:2])
    n1 = np.tanh(gx[2][:, 1] + r1 * (gh[2] + cb[0:64, 3:4]) + cb[0:64, 2:3])
    h2 = h1 + z1 * (n1 - h1)
    ca = np.maximum(h2, 0)
    ca1 = np.maximum(m["cm1"].T @ ca + cb[0:64, 4:5], 0)
    ca2 = np.maximum(m["cm2"].T @ ca1 + cb[0:64, 5:6], 0)
    q3 = m["cm3"].T @ ca2
    f1 = np.maximum(m["clm1"].T @ xi[:, 1] + cb[:, 6:7], 0)
    f2 = np.maximum(m["clm2"].T @ f1 + cb[:, 7:8], 0)
    f3 = m["clm3"].T @ f2
    q_out = (q3 + f3 + cb[0, 8])[0]
    return {"a_out": a_out, "q_out": q_out.astype(f32)}


# ----------------------------------------------------------------------------
# Bass device program
# ----------------------------------------------------------------------------

_CACHE = {}


def _build_program():
    import concourse.bass as bass  # noqa: F401
    import concourse.mybir as mybir
    import concourse.tile as tile
    from concourse import bacc
    from contextlib import ExitStack

    f32 = mybir.dt.float32
    f32r = mybir.dt.float32r
    bf16 = mybir.dt.bfloat16

    nc = bacc.Bacc("TRN2", target_bir_lowering=False, debug=False,
                   num_devices=N_CORES)
    mdt = f32r if USE_FP32R else f32     # gx matmul operand dtype
    edt = bf16 if ELT_BF16 else mdt      # elementwise-chain / recurrent dtype

    d_xp = nc.dram_tensor("xp", (PAIRS, 12, T, TB), mdt, kind="ExternalInput")
    d_wih = nc.dram_tensor("wih", (12, PAIRS, 3, 128), mdt,
                           kind="ExternalInput")
    d_whh = nc.dram_tensor("whh", (128, PAIRS, 3, 128), edt,
                           kind="ExternalInput")
    d_w12 = nc.dram_tensor("w12", (128, PAIRS, 2, 128), edt,
                           kind="ExternalInput")
    d_w3 = nc.dram_tensor("w3", (128, PAIRS, 2), edt, kind="ExternalInput")
    d_bias = nc.dram_tensor("bias", (128, PAIRS, 6), f32,
                            kind="ExternalInput")
    d_xib = nc.dram_tensor("xib", (35, T, BL), mdt, kind="ExternalInput")
    d_b3g = nc.dram_tensor("b3g", (35, T, BL), f32, kind="ExternalInput")
    d_cwih = nc.dram_tensor("cwih", (35, 3, 64), mdt, kind="ExternalInput")
    d_cwhh = nc.dram_tensor("cwhh", (64, 3, 64), mdt, kind="ExternalInput")
    d_cb = nc.dram_tensor("cb", (128, 9), f32, kind="ExternalInput")
    d_clm1 = nc.dram_tensor("clm1", (35, 128), mdt, kind="ExternalInput")
    d_clm2 = nc.dram_tensor("clm2", (128, 128), mdt, kind="ExternalInput")
    d_clm3 = nc.dram_tensor("clm3", (128, 1), mdt, kind="ExternalInput")
    d_cm1 = nc.dram_tensor("cm1", (64, 64), mdt, kind="ExternalInput")
    d_cm2 = nc.dram_tensor("cm2", (64, 64), mdt, kind="ExternalInput")
    d_cm3 = nc.dram_tensor("cm3", (64, 1), mdt, kind="ExternalInput")

    d_aout = nc.dram_tensor("a_out", (TASKS_PER_CORE * TB * T,), f32,
                            kind="ExternalOutput")
    d_qout = nc.dram_tensor("q_out", (BL,), f32, kind="ExternalOutput")
    d_dummy = nc.dram_tensor("pad_sink", (TB, 2), f32, kind="Internal")

    Sig = mybir.ActivationFunctionType.Sigmoid
    Tanh = mybir.ActivationFunctionType.Tanh
    Relu = mybir.ActivationFunctionType.Relu
    ADD = mybir.AluOpType.add
    MAX = mybir.AluOpType.max

    with tile.TileContext(nc) as tc, ExitStack() as ctx:
        res = ctx.enter_context(tc.tile_pool(name="res", bufs=1))
        s_wih = res.tile([12, PAIRS, 3, 128], mdt)
        s_whh = res.tile([128, PAIRS, 3, 128], edt)
        s_w12 = res.tile([128, PAIRS, 2, 128], edt)
        s_w3 = res.tile([128, PAIRS, 2], edt)
        s_bias = res.tile([128, PAIRS, 6], f32)
        for s, d in ((s_wih, d_wih), (s_whh, d_whh), (s_w12, d_w12),
                     (s_w3, d_w3), (s_bias, d_bias)):
            nc.sync.dma_start(out=s, in_=d.ap())

        av3 = d_aout.ap().rearrange("(j b two) -> j b two", b=TB, two=2)
        av200 = d_aout.ap().rearrange("(r c) -> r c", c=200)

        with tc.tile_pool(name="wk", bufs=3) as wk, \
             tc.tile_pool(name="pr", bufs=1, space="PSUM") as pr, \
             tc.tile_pool(name="pz", bufs=1, space="PSUM") as pz, \
             tc.tile_pool(name="pn", bufs=2, space="PSUM") as pn, \
             tc.tile_pool(name="pg", bufs=1, space="PSUM") as pg, \
             tc.tile_pool(name="pm", bufs=1, space="PSUM") as pm, \
             tc.tile_pool(name="p3", bufs=1, space="PSUM") as p3:
            for p in range(PAIRS):
                w_r = s_wih[:, p, 0, :]
                w_z = s_wih[:, p, 1, :]
                w_n = s_wih[:, p, 2, :]
                b_r = s_bias[:, p, 0:1]
                b_z = s_bias[:, p, 1:2]
                b_tn = s_bias[:, p, 2:3]
                b_hn = s_bias[:, p, 3:4]
                b_1 = s_bias[:, p, 4:5]
                b_2 = s_bias[:, p, 5:6]
                ps3 = p3.tile([128, CH], f32, tag="p3")
                for c in range(2):
                    x_t = wk.tile([12, T, CH], mdt, tag="x")
                    nc.sync.dma_start(
                        out=x_t, in_=d_xp.ap()[p, :, :, CH * c:CH * c + CH])
                    ps_r0 = pr.tile([128, CH], f32, tag="pr")
                    ps_z0 = pz.tile([128, CH], f32, tag="pz")
                    ps_n0 = pn.tile([128, CH], f32, tag="pn")
                    nc.tensor.matmul(ps_r0, w_r, x_t[:, 0, :],
                                     start=True, stop=True)
                    nc.tensor.matmul(ps_z0, w_z, x_t[:, 0, :],
                                     start=True, stop=True)
                    nc.tensor.matmul(ps_n0, w_n, x_t[:, 0, :],
                                     start=True, stop=True)
                    r0 = wk.tile([128, CH], edt, tag="r0")
                    z0 = wk.tile([128, CH], edt, tag="z0")
                    nc.scalar.activation(r0, ps_r0, Sig, bias=b_r)
                    nc.scalar.activation(z0, ps_z0, Sig, bias=b_z)
                    rbn = wk.tile([128, CH], edt, tag="rbn")
                    nc.vector.tensor_scalar_mul(rbn, r0, b_hn)
                    pre0 = wk.tile([128, CH], f32, tag="pre0")
                    nc.vector.tensor_add(pre0, ps_n0, rbn)
                    n0 = wk.tile([128, CH], edt, tag="n0")
                    nc.scalar.activation(n0, pre0, Tanh, bias=b_tn)
                    h1 = wk.tile([128, CH], edt, tag="h1")
                    nc.vector.tensor_mul(h1, z0, n0)

                    ps_r1 = pr.tile([128, CH], f32, tag="pr")
                    ps_z1 = pz.tile([128, CH], f32, tag="pz")
                    ps_n1 = pn.tile([128, CH], f32, tag="pn")
                    ps_gn = pg.tile([128, CH], f32, tag="pg")
                    nc.tensor.matmul(ps_r1, w_r, x_t[:, 1, :],
                                     start=True, stop=False)
                    nc.tensor.matmul(ps_z1, w_z, x_t[:, 1, :],
                                     start=True, stop=False)
                    nc.tensor.matmul(ps_n1, w_n, x_t[:, 1, :],
                                     start=True, stop=True)
                    nc.tensor.matmul(ps_r1, s_whh[:, p, 0, :], h1,
                                     start=False, stop=True)
                    nc.tensor.matmul(ps_z1, s_whh[:, p, 1, :], h1,
                                     start=False, stop=True)
                    nc.tensor.matmul(ps_gn, s_whh[:, p, 2, :], h1,
                                     start=True, stop=True)
                    r1 = wk.tile([128, CH], edt, tag="r0")
                    z1 = wk.tile([128, CH], edt, tag="z0")
                    nc.scalar.activation(r1, ps_r1, Sig, bias=b_r)
                    nc.scalar.activation(z1, ps_z1, Sig, bias=b_z)
                    hn = wk.tile([128, CH], edt, tag="rbn")
                    nc.vector.tensor_scalar_add(hn, ps_gn, b_hn)
                    rhn = wk.tile([128, CH], edt, tag="rhn")
                    nc.vector.tensor_mul(rhn, r1, hn)
                    pre1 = wk.tile([128, CH], f32, tag="pre0")
                    nc.vector.tensor_add(pre1, ps_n1, rhn)
                    n1 = wk.tile([128, CH], edt, tag="n0")
                    nc.scalar.activation(n1, pre1, Tanh, bias=b_tn)
                    dd = wk.tile([128, CH], edt, tag="dd")
                    nc.vector.tensor_sub(dd, n1, h1)
                    zd = wk.tile([128, CH], edt, tag="zd")
                    nc.vector.tensor_mul(zd, z1, dd)
                    h2 = wk.tile([128, CH], edt, tag="h1b")
                    nc.vector.tensor_add(h2, h1, zd)

                    # both relu planes into one [128, 1024] tile (gpsimd)
                    a01 = wk.tile([128, 2 * CH], edt, tag="a01")
                    nc.gpsimd.tensor_scalar_max(a01[:, 0:CH], h1, 0.0)
                    nc.gpsimd.tensor_scalar_max(a01[:, CH:2 * CH], h2, 0.0)

                    ps_m1 = pm.tile([128, 2 * CH], f32, tag="pm")
                    nc.tensor.matmul(ps_m1[:, 0:CH], s_w12[:, p, 0, :],
                                     a01[:, 0:CH], start=True, stop=True)
                    nc.tensor.matmul(ps_m1[:, CH:2 * CH], s_w12[:, p, 0, :],
                                     a01[:, CH:2 * CH], start=True, stop=True)
                    a1 = wk.tile([128, 2 * CH], edt, tag="a1")
                    nc.scalar.activation(a1, ps_m1, Relu, bias=b_1)
                    ps_m2 = pm.tile([128, 2 * CH], f32, tag="pm")
                    nc.tensor.matmul(ps_m2[:, 0:CH], s_w12[:, p, 1, :],
                                     a1[:, 0:CH], start=True, stop=True)
                    nc.tensor.matmul(ps_m2[:, CH:2 * CH], s_w12[:, p, 1, :],
                                     a1[:, CH:2 * CH], start=True, stop=True)
                    a2 = wk.tile([128, 2 * CH], edt, tag="a2")
                    nc.vector.tensor_scalar(a2, ps_m2, b_2, 0.0, ADD, MAX)
                    for t in range(2):
                        ofs = 32 * (2 * c + t)
                        nc.tensor.matmul(ps3[ofs:ofs + 2, :], s_w3[:, p, :],
                                         a2[:, CH * t:CH * t + CH],
                                         start=True, stop=True,
                                         tile_position=(0, ofs))

                # one PSUM->SBUF move per pair covering all 4 (c, t) planes
                sg = wk.tile([98, CH], f32, tag="sg")
                nc.vector.tensor_copy(out=sg, in_=ps3[0:98, :])
                for c in range(2):
                    for t in range(2):
                        ofs = 32 * (2 * c + t)
                        src = sg[ofs:ofs + 2, :]
                        if p < PAIRS - 1:
                            nc.sync.dma_start(
                                out=av3[2 * p:2 * p + 2,
                                        CH * c:CH * c + CH, t],
                                in_=src)
                        else:
                            nc.sync.dma_start(
                                out=av3[2 * p:2 * p + 1,
                                        CH * c:CH * c + CH, t],
                                in_=src[0:1, :])
                            nc.sync.dma_start(
                                out=d_dummy.ap()[CH * c:CH * c + CH, t],
                                in_=src[1:2, :])

        # ------------------ critic ------------------
        with tc.tile_pool(name="cw", bufs=1) as cwp, \
             tc.tile_pool(name="ck", bufs=2) as ck, \
             tc.tile_pool(name="cp", bufs=1, space="PSUM") as cp, \
             tc.tile_pool(name="cpf", bufs=1, space="PSUM") as cpf:
            s_cwih = cwp.tile([35, 3, 64], mdt)
            s_cwhh = cwp.tile([64, 3, 64], mdt)
            s_cb = cwp.tile([128, 9], f32)
            s_b3g = cwp.tile([35, T, BL], f32)
            s_clm1 = cwp.tile([35, 128], mdt)
            s_clm2 = cwp.tile([128, 128], mdt)
            s_clm3 = cwp.tile([128, 1], mdt)
            s_cm1 = cwp.tile([64, 64], mdt)
            s_cm2 = cwp.tile([64, 64], mdt)
            s_cm3 = cwp.tile([64, 1], mdt)
            for s, d in ((s_cwih, d_cwih), (s_cwhh, d_cwhh), (s_cb, d_cb),
                         (s_b3g, d_b3g),
                         (s_clm1, d_clm1), (s_clm2, d_clm2), (s_clm3, d_clm3),
                         (s_cm1, d_cm1), (s_cm2, d_cm2), (s_cm3, d_cm3)):
                nc.sync.dma_start(out=s, in_=d.ap())
            xi = cwp.tile([35, T, BL], mdt)
            nc.sync.dma_start(out=xi[0:32], in_=d_xib.ap()[0:32])
            for s, n in enumerate(_PNS):
                row = _PSLOT_ROWS[s]
                for t in range(2):
                    nc.gpsimd.dma_start(
                        out=xi[row:row + 1, t, :],
                        in_=av200[:, 100 * t + n:100 * t + n + 1])
            # p-rows: sig(raw + b3) in place (rows 32-34, both t)
            ptmp = ck.tile([35, T, BL], f32, tag="ptmp")
            nc.vector.tensor_add(ptmp[32:35], xi[32:35], s_b3g[32:35])
            nc.scalar.activation(xi[32:35], ptmp[32:35], Sig)

            cb_r = s_cb[0:64, 0:1]
            cb_z = s_cb[0:64, 1:2]
            cb_tn = s_cb[0:64, 2:3]
            cb_hn = s_cb[0:64, 3:4]

            ps_cr0 = cp.tile([64, BL], f32, tag="cr")
            ps_cz0 = cp.tile([64, BL], f32, tag="cz")
            ps_cn0 = cp.tile([64, BL], f32, tag="cn")
            nc.tensor.matmul(ps_cr0, s_cwih[:, 0, :], xi[:, 0, :],
                             start=True, stop=True)
            nc.tensor.matmul(ps_cz0, s_cwih[:, 1, :], xi[:, 0, :],
                             start=True, stop=True)
            nc.tensor.matmul(ps_cn0, s_cwih[:, 2, :], xi[:, 0, :],
                             start=True, stop=True)
            cr0 = ck.tile([64, BL], f32, tag="cr0")
            cz0 = ck.tile([64, BL], f32, tag="cz0")
            nc.scalar.activation(cr0, ps_cr0, Sig, bias=cb_r)
            nc.scalar.activation(cz0, ps_cz0, Sig, bias=cb_z)
            crbn = ck.tile([64, BL], f32, tag="crbn")
            nc.vector.tensor_scalar_mul(crbn, cr0, cb_hn)
            cpre0 = ck.tile([64, BL], f32, tag="cpre0")
            nc.vector.tensor_add(cpre0, ps_cn0, crbn)
            cn0 = ck.tile([64, BL], f32, tag="cn0")
            nc.scalar.activation(cn0, cpre0, Tanh, bias=cb_tn)
            ch1 = ck.tile([64, BL], mdt, tag="ch1")
            nc.vector.tensor_mul(ch1, cz0, cn0)

            ps_cr1 = cp.tile([64, BL], f32, tag="cr")
            ps_cz1 = cp.tile([64, BL], f32, tag="cz")
            ps_cn1 = cp.tile([64, BL], f32, tag="cn")
            ps_cg = cp.tile([64, BL], f32, tag="cg")
            nc.tensor.matmul(ps_cr1, s_cwih[:, 0, :], xi[:, 1, :],
                             start=True, stop=False)
            nc.tensor.matmul(ps_cz1, s_cwih[:, 1, :], xi[:, 1, :],
                             start=True, stop=False)
            nc.tensor.matmul(ps_cn1, s_cwih[:, 2, :], xi[:, 1, :],
                             start=True, stop=True)
            nc.tensor.matmul(ps_cr1, s_cwhh[:, 0, :], ch1,
                             start=False, stop=True)
            nc.tensor.matmul(ps_cz1, s_cwhh[:, 1, :], ch1,
                             start=False, stop=True)
            nc.tensor.matmul(ps_cg, s_cwhh[:, 2, :], ch1,
                             start=True, stop=True)
            cr1 = ck.tile([64, BL], f32, tag="cr0")
            cz1 = ck.tile([64, BL], f32, tag="cz0")
            nc.scalar.activation(cr1, ps_cr1, Sig, bias=cb_r)
            nc.scalar.activation(cz1, ps_cz1, Sig, bias=cb_z)
            chn = ck.tile([64, BL], f32, tag="crbn")
            nc.vector.tensor_scalar_add(chn, ps_cg, cb_hn)
            crhn = ck.tile([64, BL], f32, tag="crhn")
            nc.vector.tensor_mul(crhn, cr1, chn)
            cpre1 = ck.tile([64, BL], f32, tag="cpre0")
            nc.vector.tensor_add(cpre1, ps_cn1, crhn)
            cn1 = ck.tile([64, BL], f32, tag="cn0")
            nc.scalar.activation(cn1, cpre1, Tanh, bias=cb_tn)
            cdd = ck.tile([64, BL], f32, tag="cdd")
            nc.vector.tensor_sub(cdd, cn1, ch1)
            czd = ck.tile([64, BL], f32, tag="czd")
            nc.vector.tensor_mul(czd, cz1, cdd)
            ch2 = ck.tile([64, BL], f32, tag="ch2")
            nc.vector.tensor_add(ch2, ch1, czd)
            ca = ck.tile([64, BL], mdt, tag="ca")
            nc.vector.tensor_scalar_max(ca, ch2, 0.0)

            ps_cm1 = cp.tile([64, BL], f32, tag="cm")
            nc.tensor.matmul(ps_cm1, s_cm1, ca, start=True, stop=True)
            ca1 = ck.tile([64, BL], mdt, tag="ca1")
            nc.scalar.activation(ca1, ps_cm1, Relu, bias=s_cb[0:64, 4:5])
            ps_cm2 = cp.tile([64, BL], f32, tag="cm")
            nc.tensor.matmul(ps_cm2, s_cm2, ca1, start=True, stop=True)
            ca2 = ck.tile([64, BL], mdt, tag="ca2")
            nc.scalar.activation(ca2, ps_cm2, Relu, bias=s_cb[0:64, 5:6])
            ps_cq = cpf.tile([1, BL], f32, tag="cq")
            nc.tensor.matmul(ps_cq, s_cm3, ca2, start=True, stop=False)

            ps_cf1 = cp.tile([128, BL], f32, tag="cf")
            nc.tensor.matmul(ps_cf1, s_clm1, xi[:, 1, :],
                             start=True, stop=True)
            cf1 = ck.tile([128, BL], mdt, tag="cf1")
            nc.scalar.activation(cf1, ps_cf1, Relu, bias=s_cb[:, 6:7])
            ps_cf2 = cp.tile([128, BL], f32, tag="cf")
            nc.tensor.matmul(ps_cf2, s_clm2, cf1, start=True, stop=True)
            cf2 = ck.tile([128, BL], mdt, tag="cf2")
            nc.scalar.activation(cf2, ps_cf2, Relu, bias=s_cb[:, 7:8])
            nc.tensor.matmul(ps_cq, s_clm3, cf2, start=False, stop=True)

            qout = ck.tile([1, BL], f32, tag="qout")
            nc.vector.tensor_scalar_add(qout, ps_cq, s_cb[0:1, 8:9])
            nc.sync.dma_start(out=d_qout.ap(), in_=qout)

    nc.compile()
    return nc


def kernel(**inputs):
    in_maps, aw = _prep(inputs)
    if os.environ.get("BASS_KERNEL_EMULATE", ""):
        results = [_emulate_core(m) for m in in_maps]
        return _post(results, aw)

    if "nc" not in _CACHE:
        _CACHE["nc"] = _build_program()
    nc = _CACHE["nc"]

    from concourse import bass_utils
    trace = bool(os.environ.get("BASS_KERNEL_TRACE", ""))
    tmpdir = os.environ.get("BASS_KERNEL_TMPDIR") or None
    r = bass_utils.run_bass_kernel_spmd(
        nc, in_maps, core_ids=list(range(N_CORES)), trace=trace,
        tmpdir=tmpdir)
    _CACHE["last_exec_time_ns"] = r.exec_time_ns
    return _post(r.results, aw)
